# revision 4
# baseline (speedup 1.0000x reference)
"""MoE layer (top-2 of 8 experts) on 8 Trainium2 NeuronCores.

Strategy: expert parallelism. The router (tiny: [T,D]@[D,E] + softmax/top-2)
runs on host; tokens are gathered per expert, padded to a common capacity C,
and each core runs one expert's FFN:

    yT_e = W2_e.T @ gelu(W1_e.T @ xT_e + b1_e) + b2_e      (feature-major)

Device matmuls use float32r (full-rate fp32 on the PE array; ~1e-3 rel err)
with fp32 PSUM accumulation, exact-erf GELU on the scalar engine with the b1
bias fused in. The host scatter-adds the two weighted expert outputs per
token (mathematically identical to the dense reference since combine weights
are zero for unselected experts).

Per-core loop: tokens are processed in super-chunks of SC=768 columns so the
intermediate hT [4096, SC] stays in SBUF; weights stream through a shared
double-buffered pool. PSUM holds [128, <=512] accumulation tiles.
"""

import numpy as np

import concourse.tile as tile
import concourse.mybir as mybir
from concourse import bacc
from concourse.bass_utils import run_bass_kernel_spmd

E = 8
TOP_K = 2
D = 1024
H = 4096
DC = D // 128  # 8 d-chunks
HC = H // 128  # 32 h-chunks
SC = 768  # tokens per super-chunk (hT SBUF residency)
DT = mybir.dt.float32r  # matmul operand dtype on device

_cache: dict[int, object] = {}


def _chunks(total, size):
    out = []
    base = 0
    while base < total:
        out.append((base, min(size, total - base)))
        base += size
    return out


def _build(C, repeat=1):
    """One SPMD program: expert FFN over C (padded) tokens.

    repeat>1 re-runs the identical compute (overwriting yt) — used only by
    the test harness to difference out per-call dispatch overhead.
    """
    if (C, repeat) in _cache:
        return _cache[(C, repeat)]
    nc = bacc.Bacc("TRN2", target_bir_lowering=False, num_devices=E)

    xt_d = nc.declare_dram_parameter("xt", [D, C], DT, isOutput=False)
    w1_d = nc.declare_dram_parameter("w1", [HC, 128, DC, 128], DT, isOutput=False)
    b1_d = nc.declare_dram_parameter("b1", [128, HC], mybir.dt.float32, isOutput=False)
    w2_d = nc.declare_dram_parameter("w2", [DC, 128, HC, 128], DT, isOutput=False)
    b2_d = nc.declare_dram_parameter("b2", [128, DC], mybir.dt.float32, isOutput=False)
    yt_d = nc.declare_dram_parameter("yt", [D, C], mybir.dt.float32, isOutput=True)

    with tile.TileContext(nc) as tc:
        with (
            tc.tile_pool(name="bias", bufs=1) as bp,
            tc.tile_pool(name="x", bufs=1) as xp,
            tc.tile_pool(name="h", bufs=1) as hp,
            tc.tile_pool(name="w", bufs=2) as wp,
            tc.tile_pool(name="o", bufs=4) as op,
            tc.tile_pool(name="ps", bufs=8, space="PSUM") as pp,
        ):
            b1t = bp.tile([128, HC], mybir.dt.float32)
            nc.sync.dma_start(b1t[:], b1_d[:])
            b2t = bp.tile([128, DC], mybir.dt.float32)
            nc.sync.dma_start(b2t[:], b2_d[:])

            for _rep in range(repeat):
              for sbase, sw in _chunks(C, SC):
                cols = _chunks(sw, 512)

                # ---- load this super-chunk of tokens (feature-major) ----
                xt = xp.tile([128, DC * SC], DT, tag="x")
                for di in range(DC):
                    nc.sync.dma_start(
                        xt[:, di * sw : (di + 1) * sw],
                        xt_d[di * 128 : (di + 1) * 128, sbase : sbase + sw],
                    )

                ht = hp.tile([128, HC * SC], DT, tag="h")

                # ---- phase A: hT = gelu(W1.T @ xT + b1) ----
                for hj in range(HC):
                    w1t = wp.tile([128, DC * 128], DT, tag="w")
                    nc.sync.dma_start(
                        w1t[:], w1_d[hj].rearrange("d di h -> d (di h)")
                    )
                    for cbase, cw in cols:
                        ps = pp.tile([128, cw], mybir.dt.float32, tag="ps")
                        for di in range(DC):
                            nc.tensor.matmul(
                                ps[:],
                                w1t[:, di * 128 : (di + 1) * 128],
                                xt[:, di * sw + cbase : di * sw + cbase + cw],
                                start=(di == 0),
                                stop=(di == DC - 1),
                            )
                        nc.scalar.activation(
                            ht[:, hj * sw + cbase : hj * sw + cbase + cw],
                            ps[:],
                            mybir.ActivationFunctionType.Gelu,
                            bias=b1t[:, hj : hj + 1],
                        )

                # ---- phase B: yT = W2.T @ hT + b2 ----
                for dj in range(DC):
                    w2t = wp.tile([128, HC * 128], DT, tag="w")
                    nc.sync.dma_start(
                        w2t[:], w2_d[dj].rearrange("h hj d -> h (hj d)")
                    )
                    for cbase, cw in cols:
                        ps2 = pp.tile([128, cw], mybir.dt.float32, tag="ps")
                        for hj in range(HC):
                            nc.tensor.matmul(
                                ps2[:],
                                w2t[:, hj * 128 : (hj + 1) * 128],
                                ht[:, hj * sw + cbase : hj * sw + cbase + cw],
                                start=(hj == 0),
                                stop=(hj == HC - 1),
                            )
                        ot = op.tile([128, cw], mybir.dt.float32, tag="o")
                        nc.scalar.activation(
                            ot[:],
                            ps2[:],
                            mybir.ActivationFunctionType.Identity,
                            bias=b2t[:, dj : dj + 1],
                        )
                        nc.sync.dma_start(
                            yt_d[
                                dj * 128 : (dj + 1) * 128,
                                sbase + cbase : sbase + cbase + cw,
                            ],
                            ot[:],
                        )
    nc.compile()
    _cache[(C, repeat)] = nc
    return nc


def _route(xf, Wg):
    """Host router: softmax top-2, normalized weights, aux loss."""
    logits = xf @ Wg  # [T, E] fp32
    m = logits.max(axis=1, keepdims=True)
    ex = np.exp(logits - m)
    probs = ex / ex.sum(axis=1, keepdims=True)
    idx = np.argsort(-probs, axis=1, kind="stable")[:, :TOP_K]  # [T, K]
    w = np.take_along_axis(probs, idx, axis=1)
    w = w / w.sum(axis=1, keepdims=True)

    T = xf.shape[0]
    counts = np.zeros((T, E), dtype=np.float32)
    np.put_along_axis(counts, idx, 1.0, axis=1)
    f_i = counts.mean(axis=0)
    p_i = probs.mean(axis=0, dtype=np.float32)
    aux_loss = np.float32(E) * np.sum(f_i * p_i, dtype=np.float32)
    return idx, w.astype(np.float32), aux_loss


def kernel(x, Wg, W1, b1, W2, b2):
    x = np.asarray(x, dtype=np.float32)
    Wg = np.asarray(Wg, dtype=np.float32)
    W1 = np.asarray(W1, dtype=np.float32)
    b1 = np.asarray(b1, dtype=np.float32)
    W2 = np.asarray(W2, dtype=np.float32)
    b2 = np.asarray(b2, dtype=np.float32)

    B, S, _ = x.shape
    T = B * S
    xf = x.reshape(T, D)

    idx, w, aux_loss = _route(xf, Wg)

    # token lists per expert
    toks = [np.where((idx == e).any(axis=1))[0] for e in range(E)]
    n_max = max(len(t) for t in toks)
    C = max(256, -(-n_max // 256) * 256)

    nc = _build(C)

    in_maps = []
    for e in range(E):
        te = toks[e]
        xT = np.zeros((D, C), dtype=np.float32)
        xT[:, : len(te)] = xf[te].T
        w1p = np.ascontiguousarray(
            W1[e].reshape(DC, 128, HC, 128).transpose(2, 1, 0, 3)
        )
        w2p = np.ascontiguousarray(
            W2[e].reshape(HC, 128, DC, 128).transpose(2, 1, 0, 3)
        )
        b1p = np.ascontiguousarray(b1[e].reshape(HC, 128).T)
        b2p = np.ascontiguousarray(b2[e].reshape(DC, 128).T)
        in_maps.append(
            {"xt": xT, "w1": w1p, "b1": b1p, "w2": w2p, "b2": b2p}
        )

    res = run_bass_kernel_spmd(nc, in_maps, core_ids=list(range(E)))

    out = np.zeros((T, D), dtype=np.float32)
    for e in range(E):
        te = toks[e]
        if len(te) == 0:
            continue
        y_e = res.results[e]["yt"][:, : len(te)].T  # [n_e, D]
        # weight of expert e for each of its tokens
        sel = idx[te] == e  # [n_e, K] one-hot over K
        w_e = (w[te] * sel).sum(axis=1).astype(np.float32)  # [n_e]
        out[te] += w_e[:, None] * y_e

    return out.reshape(B, S, D), aux_loss


# revision 13
# speedup vs baseline: 1.1966x; 1.1966x over previous
"""MoE layer (top-2 of 8 experts) on 8 Trainium2 NeuronCores.

Strategy: expert parallelism. The router (tiny: [T,D]@[D,E] + softmax/top-2)
runs on host; tokens are gathered per expert, padded to a common capacity C,
and each core runs one expert's FFN in feature-major layout:

    yT_e = W2_e.T @ gelu(W1_e.T @ xT_e + b1_e)        (b2 added on host)

Device matmuls use float32r (full-rate fp32 on the PE array, ~1e-3 rel err)
with fp32 PSUM accumulation and exact-erf GELU on the scalar engine (b1 bias
fused). The host scatter-adds the two weighted expert outputs per token —
mathematically identical to the dense reference since combine weights are
zero for unselected experts.

Per-core loop: tokens stream in super-chunks of SC=768 columns so the
intermediate hT [4096, SC] stays in SBUF. Weight tiles stream through
triple-buffered pools on the SP DMA queue; output stores DMA straight from
PSUM on the scalar queue so they never block weight loads (head-of-line).
"""

import numpy as np

import concourse.tile as tile
import concourse.mybir as mybir
from concourse import bacc
from concourse.bass_utils import run_bass_kernel_spmd

E = 8
TOP_K = 2
D = 1024
H = 4096
DC = D // 128  # 8 d-chunks

HC = H // 128  # 32 h-chunks
SC = 768  # tokens per super-chunk (hT SBUF residency)
DT = mybir.dt.float32r  # matmul operand dtype on device

_cache: dict[tuple, object] = {}


def _chunks(total, size):
    out = []
    base = 0
    while base < total:
        out.append((base, min(size, total - base)))
        base += size
    return out


def _cols(sw):
    """Split a super-chunk into matmul column blocks of <=512, keeping every
    block >=256 wide when possible (float32r runs 4x slower below 256)."""
    out = _chunks(sw, 512)
    if len(out) >= 2 and out[-1][1] < 256:
        (b0, w0), (b1, w1) = out[-2], out[-1]
        shift = 256 - w1
        out[-2] = (b0, w0 - shift)
        out[-1] = (b1 - shift, 256)
    return out


def _build(C, repeat=1):
    """One SPMD program: expert FFN over C (padded) tokens.

    repeat>1 re-runs the identical compute (overwriting yt) — used only by
    the test harness to difference out per-call dispatch overhead.
    """
    if (C, repeat) in _cache:
        return _cache[(C, repeat)]
    nc = bacc.Bacc("TRN2", target_bir_lowering=False, num_devices=E)

    xt_d = nc.declare_dram_parameter("xt", [D, C], DT, isOutput=False)
    w1_d = nc.declare_dram_parameter("w1", [HC, 128, DC, 128], DT, isOutput=False)
    b1_d = nc.declare_dram_parameter("b1", [128, HC], mybir.dt.float32, isOutput=False)
    w2_d = nc.declare_dram_parameter("w2", [DC, 128, HC, 128], DT, isOutput=False)
    yt_d = nc.declare_dram_parameter("yt", [D, C], mybir.dt.float32, isOutput=True)

    HHALF = HC // 2  # w2 streams in half-tiles of 16 h-chunks

    with tile.TileContext(nc) as tc:
        with (
            tc.tile_pool(name="bias", bufs=1) as bp,
            tc.tile_pool(name="x", bufs=2) as xp,
            tc.tile_pool(name="h", bufs=1) as hp,
            tc.tile_pool(name="w1p", bufs=3) as w1p,
            tc.tile_pool(name="w2p", bufs=3) as w2p,
            tc.tile_pool(name="o", bufs=4) as op,
            tc.tile_pool(name="psA", bufs=4, space="PSUM") as pA,
            tc.tile_pool(name="psB", bufs=4, space="PSUM") as pB,
        ):
            b1t = bp.tile([128, HC], mybir.dt.float32)
            nc.scalar.dma_start(b1t[:], b1_d[:])

            w1_tiles = {}

            def w1_prefetch(hj):
                t = w1p.tile([128, DC * 128], DT, tag="w1")
                nc.sync.dma_start(t[:], w1_d[hj].rearrange("d di h -> d (di h)"))
                w1_tiles[hj] = t

            def load_xt(sbase, sw):
                # column-major so the first column block lands fast
                xt = xp.tile([128, DC * SC], DT, tag="x")
                for cbase, cw in _cols(sw):
                    for di in range(DC):
                        nc.sync.dma_start(
                            xt[:, di * sw + cbase : di * sw + cbase + cw],
                            xt_d[
                                di * 128 : (di + 1) * 128,
                                sbase + cbase : sbase + cbase + cw,
                            ],
                        )
                return xt

            scs = _chunks(C, SC) * repeat
            w1_prefetch(0)  # ahead of the token loads in SP queue order
            xt_next = load_xt(*scs[0])

            for si, (sbase, sw) in enumerate(scs):
                    cols = _cols(sw)
                    xt = xt_next

                    # w2 half-tiles to prefetch this super-chunk, in use order
                    w2_queue = [(dj, hh) for dj in range(DC) for hh in range(2)]
                    w2_tiles = {}

                    def w2_prefetch(n=1):
                        for _ in range(n):
                            if not w2_queue:
                                return
                            dj, hh = w2_queue.pop(0)
                            t = w2p.tile([128, HHALF * 128], DT, tag="w2")
                            nc.sync.dma_start(
                                t[:],
                                w2_d[
                                    dj, :, hh * HHALF : (hh + 1) * HHALF, :
                                ].rearrange("h hj d -> h (hj d)"),
                            )
                            w2_tiles[(dj, hh)] = t

                    ht = hp.tile([128, HC * SC], DT, tag="h")

                    # ---- phase A: hT = gelu(W1.T @ xT + b1) ----
                    for hj in range(HC):
                        if hj not in w1_tiles:
                            w1_prefetch(hj)
                        if hj + 1 < HC and hj + 1 not in w1_tiles:
                            w1_prefetch(hj + 1)
                        w1t = w1_tiles.pop(hj)
                        if hj in (1, 11, 21):  # warm the w2 pipeline early
                            w2_prefetch()
                        for cbase, cw in cols:
                            ps = pA.tile([128, cw], mybir.dt.float32, tag="psA")
                            for di in range(DC):
                                nc.tensor.matmul(
                                    ps[:],
                                    w1t[:, di * 128 : (di + 1) * 128],
                                    xt[:, di * sw + cbase : di * sw + cbase + cw],
                                    start=(di == 0),
                                    stop=(di == DC - 1),
                                )
                            nc.scalar.activation(
                                ht[:, hj * sw + cbase : hj * sw + cbase + cw],
                                ps[:],
                                mybir.ActivationFunctionType.Gelu,
                                bias=b1t[:, hj : hj + 1],
                            )

                    # ---- phase B: yT = W2.T @ hT (stores straight from PSUM)
                    for dj in range(DC):
                        if dj == 1 and si + 1 < len(scs):
                            # prefetch next super-chunk tokens + first weights
                            # while PE is busy
                            xt_next = load_xt(*scs[si + 1])
                            w1_prefetch(0)
                        for cbase, cw in cols:
                            ps2 = pB.tile([128, cw], mybir.dt.float32, tag="psB")
                            for hj in range(HC):
                                if hj % HHALF == 0 and cbase == 0:
                                    w2_prefetch()
                                w2t = w2_tiles[(dj, hj // HHALF)]
                                nc.tensor.matmul(
                                    ps2[:],
                                    w2t[
                                        :,
                                        (hj % HHALF) * 128 : (hj % HHALF + 1) * 128,
                                    ],
                                    ht[:, hj * sw + cbase : hj * sw + cbase + cw],
                                    start=(hj == 0),
                                    stop=(hj == HC - 1),
                                )
                            ot = op.tile([128, cw], mybir.dt.float32, tag="o")
                            nc.vector.tensor_copy(ot[:], ps2[:])
                            nc.scalar.dma_start(
                                yt_d[
                                    dj * 128 : (dj + 1) * 128,
                                    sbase + cbase : sbase + cbase + cw,
                                ],
                                ot[:],
                            )
    nc.compile()
    _cache[(C, repeat)] = nc
    return nc


def _route(xf, Wg):
    """Host router: softmax top-2, normalized weights, aux loss."""
    logits = xf @ Wg  # [T, E] fp32
    m = logits.max(axis=1, keepdims=True)
    ex = np.exp(logits - m)
    probs = ex / ex.sum(axis=1, keepdims=True)
    idx = np.argsort(-probs, axis=1, kind="stable")[:, :TOP_K]  # [T, K]
    w = np.take_along_axis(probs, idx, axis=1)
    w = w / w.sum(axis=1, keepdims=True)

    T = xf.shape[0]
    counts = np.zeros((T, E), dtype=np.float32)
    np.put_along_axis(counts, idx, 1.0, axis=1)
    f_i = counts.mean(axis=0)
    p_i = probs.mean(axis=0, dtype=np.float32)
    aux_loss = np.float32(E) * np.sum(f_i * p_i, dtype=np.float32)
    return idx, w.astype(np.float32), aux_loss


def kernel(x, Wg, W1, b1, W2, b2):
    x = np.asarray(x, dtype=np.float32)
    Wg = np.asarray(Wg, dtype=np.float32)
    W1 = np.asarray(W1, dtype=np.float32)
    b1 = np.asarray(b1, dtype=np.float32)
    W2 = np.asarray(W2, dtype=np.float32)
    b2 = np.asarray(b2, dtype=np.float32)

    B, S, _ = x.shape
    T = B * S
    xf = x.reshape(T, D)

    idx, w, aux_loss = _route(xf, Wg)

    # token lists per expert
    toks = [np.where((idx == e).any(axis=1))[0] for e in range(E)]
    n_max = max(len(t) for t in toks)
    C = max(256, -(-n_max // 64) * 64)

    nc = _build(C)

    in_maps = []
    for e in range(E):
        te = toks[e]
        xT = np.zeros((D, C), dtype=np.float32)
        xT[:, : len(te)] = xf[te].T
        w1p = np.ascontiguousarray(
            W1[e].reshape(DC, 128, HC, 128).transpose(2, 1, 0, 3)
        )
        w2p = np.ascontiguousarray(
            W2[e].reshape(HC, 128, DC, 128).transpose(2, 1, 0, 3)
        )
        b1p = np.ascontiguousarray(b1[e].reshape(HC, 128).T)
        in_maps.append({"xt": xT, "w1": w1p, "b1": b1p, "w2": w2p})

    res = run_bass_kernel_spmd(nc, in_maps, core_ids=list(range(E)))

    out = np.zeros((T, D), dtype=np.float32)
    for e in range(E):
        te = toks[e]
        if len(te) == 0:
            continue
        y_e = res.results[e]["yt"][:, : len(te)].T + b2[e][None, :]  # [n_e, D]
        # weight of expert e for each of its tokens
        sel = idx[te] == e  # [n_e, K] one-hot over K
        w_e = (w[te] * sel).sum(axis=1).astype(np.float32)  # [n_e]
        out[te] += w_e[:, None] * y_e

    return out.reshape(B, S, D), aux_loss


# revision 14
# speedup vs baseline: 1.2308x; 1.0286x over previous
"""MoE layer (top-2 of 8 experts) on 8 Trainium2 NeuronCores.

Strategy: expert parallelism. The router (tiny: [T,D]@[D,E] + softmax/top-2)
runs on host; tokens are gathered per expert, padded to a common capacity C,
and each core runs one expert's FFN in feature-major layout:

    yT_e = W2_e.T @ gelu(W1_e.T @ xT_e + b1_e)        (b2 added on host)

Device matmuls use float32r (full-rate fp32 on the PE array, ~1e-3 rel err)
with fp32 PSUM accumulation and exact-erf GELU on the scalar engine (b1 bias
fused). The host scatter-adds the two weighted expert outputs per token —
mathematically identical to the dense reference since combine weights are
zero for unselected experts.

Per-core loop: tokens stream in super-chunks of SC=768 columns so the
intermediate hT [4096, SC] stays in SBUF. Weight tiles stream through
triple-buffered pools on the SP DMA queue; output stores DMA straight from
PSUM on the scalar queue so they never block weight loads (head-of-line).
"""

import numpy as np

import concourse.tile as tile
import concourse.mybir as mybir
from concourse import bacc
from concourse.bass_utils import run_bass_kernel_spmd

E = 8
TOP_K = 2
D = 1024
H = 4096
DC = D // 128  # 8 d-chunks

HC = H // 128  # 32 h-chunks
DT = mybir.dt.float32r  # matmul operand dtype on device
BF16_H = False  # store hT (gelu output) + W2 in bf16: halves phase-B operand
#                 traffic and allows bigger super-chunks (fewer weight passes)


def _sc_size():
    return 1120 if BF16_H else 768


def _h_dt():
    return mybir.dt.bfloat16 if BF16_H else DT

_cache: dict[tuple, object] = {}


def _chunks(total, size):
    out = []
    base = 0
    while base < total:
        out.append((base, min(size, total - base)))
        base += size
    return out


def _cols(sw):
    """Split a super-chunk into matmul column blocks of <=512, keeping every
    block >=256 wide when possible (float32r runs 4x slower below 256)."""
    out = _chunks(sw, 512)
    if len(out) >= 2 and out[-1][1] < 256:
        (b0, w0), (b1, w1) = out[-2], out[-1]
        shift = 256 - w1
        out[-2] = (b0, w0 - shift)
        out[-1] = (b1 - shift, 256)
    return out


def _build(C, repeat=1):
    """One SPMD program: expert FFN over C (padded) tokens.

    repeat>1 re-runs the identical compute (overwriting yt) — used only by
    the test harness to difference out per-call dispatch overhead.
    """
    key = (C, repeat, BF16_H)
    if key in _cache:
        return _cache[key]
    SC = _sc_size()
    HDT = _h_dt()
    nc = bacc.Bacc("TRN2", target_bir_lowering=False, num_devices=E)

    xt_d = nc.declare_dram_parameter("xt", [D, C], DT, isOutput=False)
    w1_d = nc.declare_dram_parameter("w1", [HC, 128, DC, 128], DT, isOutput=False)
    b1_d = nc.declare_dram_parameter("b1", [128, HC], mybir.dt.float32, isOutput=False)
    w2_d = nc.declare_dram_parameter("w2", [DC, 128, HC, 128], _h_dt(), isOutput=False)
    yt_d = nc.declare_dram_parameter("yt", [D, C], mybir.dt.float32, isOutput=True)

    HHALF = HC // 2  # w2 streams in half-tiles of 16 h-chunks

    with tile.TileContext(nc) as tc:
        with (
            tc.tile_pool(name="bias", bufs=1) as bp,
            tc.tile_pool(name="x", bufs=2) as xp,
            tc.tile_pool(name="h", bufs=1) as hp,
            tc.tile_pool(name="w1p", bufs=3) as w1p,
            tc.tile_pool(name="w2p", bufs=3) as w2p,
            tc.tile_pool(name="o", bufs=4) as op,
            tc.tile_pool(name="psA", bufs=4, space="PSUM") as pA,
            tc.tile_pool(name="psB", bufs=4, space="PSUM") as pB,
        ):
            b1t = bp.tile([128, HC], mybir.dt.float32)
            nc.scalar.dma_start(b1t[:], b1_d[:])

            w1_tiles = {}

            def w1_prefetch(hj):
                t = w1p.tile([128, DC * 128], DT, tag="w1")
                nc.sync.dma_start(t[:], w1_d[hj].rearrange("d di h -> d (di h)"))
                w1_tiles[hj] = t

            def load_xt(sbase, sw):
                # column-major so the first column block lands fast
                xt = xp.tile([128, DC * SC], DT, tag="x")
                for cbase, cw in _cols(sw):
                    for di in range(DC):
                        nc.sync.dma_start(
                            xt[:, di * sw + cbase : di * sw + cbase + cw],
                            xt_d[
                                di * 128 : (di + 1) * 128,
                                sbase + cbase : sbase + cbase + cw,
                            ],
                        )
                return xt

            scs = _chunks(C, SC) * repeat
            w1_prefetch(0)  # ahead of the token loads in SP queue order
            xt_next = load_xt(*scs[0])

            for si, (sbase, sw) in enumerate(scs):
                    cols = _cols(sw)
                    xt = xt_next

                    # w2 half-tiles to prefetch this super-chunk, in use order
                    w2_queue = [(dj, hh) for dj in range(DC) for hh in range(2)]
                    w2_tiles = {}

                    def w2_prefetch(n=1):
                        for _ in range(n):
                            if not w2_queue:
                                return
                            dj, hh = w2_queue.pop(0)
                            t = w2p.tile([128, HHALF * 128], HDT, tag="w2")
                            nc.sync.dma_start(
                                t[:],
                                w2_d[
                                    dj, :, hh * HHALF : (hh + 1) * HHALF, :
                                ].rearrange("h hj d -> h (hj d)"),
                            )
                            w2_tiles[(dj, hh)] = t

                    ht = hp.tile([128, HC * SC], HDT, tag="h")

                    # ---- phase A: hT = gelu(W1.T @ xT + b1) ----
                    for hj in range(HC):
                        if hj not in w1_tiles:
                            w1_prefetch(hj)
                        if hj + 1 < HC and hj + 1 not in w1_tiles:
                            w1_prefetch(hj + 1)
                        w1t = w1_tiles.pop(hj)
                        if hj in (1, 11, 21):  # warm the w2 pipeline early
                            w2_prefetch()
                        for cbase, cw in cols:
                            ps = pA.tile([128, cw], mybir.dt.float32, tag="psA")
                            for di in range(DC):
                                nc.tensor.matmul(
                                    ps[:],
                                    w1t[:, di * 128 : (di + 1) * 128],
                                    xt[:, di * sw + cbase : di * sw + cbase + cw],
                                    start=(di == 0),
                                    stop=(di == DC - 1),
                                )
                            nc.scalar.activation(
                                ht[:, hj * sw + cbase : hj * sw + cbase + cw],
                                ps[:],
                                mybir.ActivationFunctionType.Gelu,
                                bias=b1t[:, hj : hj + 1],
                            )

                    # ---- phase B: yT = W2.T @ hT (stores straight from PSUM)
                    for dj in range(DC):
                        if dj == 1 and si + 1 < len(scs):
                            # prefetch next super-chunk tokens + first weights
                            # while PE is busy
                            xt_next = load_xt(*scs[si + 1])
                            w1_prefetch(0)
                        for cbase, cw in cols:
                            ps2 = pB.tile([128, cw], mybir.dt.float32, tag="psB")
                            for hj in range(HC):
                                if hj % HHALF == 0 and cbase == 0:
                                    w2_prefetch()
                                w2t = w2_tiles[(dj, hj // HHALF)]
                                nc.tensor.matmul(
                                    ps2[:],
                                    w2t[
                                        :,
                                        (hj % HHALF) * 128 : (hj % HHALF + 1) * 128,
                                    ],
                                    ht[:, hj * sw + cbase : hj * sw + cbase + cw],
                                    start=(hj == 0),
                                    stop=(hj == HC - 1),
                                )
                            ot = op.tile([128, cw], mybir.dt.float32, tag="o")
                            nc.vector.tensor_copy(ot[:], ps2[:])
                            nc.scalar.dma_start(
                                yt_d[
                                    dj * 128 : (dj + 1) * 128,
                                    sbase + cbase : sbase + cbase + cw,
                                ],
                                ot[:],
                            )
    nc.compile()
    _cache[key] = nc
    return nc


def _route(xf, Wg):
    """Host router: softmax top-2, normalized weights, aux loss."""
    logits = xf @ Wg  # [T, E] fp32
    m = logits.max(axis=1, keepdims=True)
    ex = np.exp(logits - m)
    probs = ex / ex.sum(axis=1, keepdims=True)
    idx = np.argsort(-probs, axis=1, kind="stable")[:, :TOP_K]  # [T, K]
    w = np.take_along_axis(probs, idx, axis=1)
    w = w / w.sum(axis=1, keepdims=True)

    T = xf.shape[0]
    counts = np.zeros((T, E), dtype=np.float32)
    np.put_along_axis(counts, idx, 1.0, axis=1)
    f_i = counts.mean(axis=0)
    p_i = probs.mean(axis=0, dtype=np.float32)
    aux_loss = np.float32(E) * np.sum(f_i * p_i, dtype=np.float32)
    return idx, w.astype(np.float32), aux_loss


def kernel(x, Wg, W1, b1, W2, b2):
    x = np.asarray(x, dtype=np.float32)
    Wg = np.asarray(Wg, dtype=np.float32)
    W1 = np.asarray(W1, dtype=np.float32)
    b1 = np.asarray(b1, dtype=np.float32)
    W2 = np.asarray(W2, dtype=np.float32)
    b2 = np.asarray(b2, dtype=np.float32)

    B, S, _ = x.shape
    T = B * S
    xf = x.reshape(T, D)

    idx, w, aux_loss = _route(xf, Wg)

    # token lists per expert
    toks = [np.where((idx == e).any(axis=1))[0] for e in range(E)]
    n_max = max(len(t) for t in toks)
    C = max(256, -(-n_max // 64) * 64)

    nc = _build(C)

    in_maps = []
    for e in range(E):
        te = toks[e]
        xT = np.zeros((D, C), dtype=np.float32)
        xT[:, : len(te)] = xf[te].T
        w1p = np.ascontiguousarray(
            W1[e].reshape(DC, 128, HC, 128).transpose(2, 1, 0, 3)
        )
        w2p = np.ascontiguousarray(
            W2[e].reshape(HC, 128, DC, 128).transpose(2, 1, 0, 3)
        )
        if BF16_H:
            import ml_dtypes

            w2p = w2p.astype(ml_dtypes.bfloat16)
        b1p = np.ascontiguousarray(b1[e].reshape(HC, 128).T)
        in_maps.append({"xt": xT, "w1": w1p, "b1": b1p, "w2": w2p})

    res = run_bass_kernel_spmd(nc, in_maps, core_ids=list(range(E)))

    out = np.zeros((T, D), dtype=np.float32)
    for e in range(E):
        te = toks[e]
        if len(te) == 0:
            continue
        y_e = res.results[e]["yt"][:, : len(te)].T + b2[e][None, :]  # [n_e, D]
        # weight of expert e for each of its tokens
        sel = idx[te] == e  # [n_e, K] one-hot over K
        w_e = (w[te] * sel).sum(axis=1).astype(np.float32)  # [n_e]
        out[te] += w_e[:, None] * y_e

    return out.reshape(B, S, D), aux_loss


# revision 15
# speedup vs baseline: 3.6944x; 3.0017x over previous
"""MoE layer (top-2 of 8 experts) on 8 Trainium2 NeuronCores.

Strategy: expert parallelism. The router (tiny: [T,D]@[D,E] + softmax/top-2)
runs on host; tokens are gathered per expert, padded to a common capacity C,
and each core runs one expert's FFN in feature-major layout:

    yT_e = W2_e.T @ gelu(W1_e.T @ xT_e + b1_e)        (b2 added on host)

Device matmuls use float32r (full-rate fp32 on the PE array, ~1e-3 rel err)
with fp32 PSUM accumulation and exact-erf GELU on the scalar engine (b1 bias
fused). The host scatter-adds the two weighted expert outputs per token —
mathematically identical to the dense reference since combine weights are
zero for unselected experts.

Per-core loop: tokens stream in super-chunks of SC=768 columns so the
intermediate hT [4096, SC] stays in SBUF. Weight tiles stream through
triple-buffered pools on the SP DMA queue; output stores DMA straight from
PSUM on the scalar queue so they never block weight loads (head-of-line).
"""

import numpy as np

import concourse.tile as tile
import concourse.mybir as mybir
from concourse import bacc
from concourse.bass_utils import run_bass_kernel_spmd

E = 8
TOP_K = 2
D = 1024
H = 4096
DC = D // 128  # 8 d-chunks

HC = H // 128  # 32 h-chunks
DT = mybir.dt.float32r  # matmul operand dtype on device
BF16_H = False  # store hT (gelu output) + W2 in bf16: halves phase-B operand
#                 traffic and allows bigger super-chunks (fewer weight passes)


def _sc_size():
    return 1120 if BF16_H else 768


def _h_dt():
    return mybir.dt.bfloat16 if BF16_H else DT

_cache: dict[tuple, object] = {}


def _chunks(total, size):
    out = []
    base = 0
    while base < total:
        out.append((base, min(size, total - base)))
        base += size
    return out


def _cols(sw):
    """Split a super-chunk into matmul column blocks of <=512, keeping every
    block >=256 wide when possible (float32r runs 4x slower below 256)."""
    out = _chunks(sw, 512)
    if len(out) >= 2 and out[-1][1] < 256:
        (b0, w0), (b1, w1) = out[-2], out[-1]
        shift = 256 - w1
        out[-2] = (b0, w0 - shift)
        out[-1] = (b1 - shift, 256)
    return out


def _build(C, repeat=1):
    """One SPMD program: expert FFN over C (padded) tokens.

    repeat>1 re-runs the identical compute (overwriting yt) — used only by
    the test harness to difference out per-call dispatch overhead.
    """
    key = (C, repeat, BF16_H)
    if key in _cache:
        return _cache[key]
    SC = _sc_size()
    HDT = _h_dt()
    nc = bacc.Bacc("TRN2", target_bir_lowering=False, num_devices=E)

    xt_d = nc.declare_dram_parameter("xt", [D, C], DT, isOutput=False)
    w1_d = nc.declare_dram_parameter("w1", [HC, 128, DC, 128], DT, isOutput=False)
    b1_d = nc.declare_dram_parameter("b1", [128, HC], mybir.dt.float32, isOutput=False)
    w2_d = nc.declare_dram_parameter("w2", [DC, 128, HC, 128], _h_dt(), isOutput=False)
    yt_d = nc.declare_dram_parameter("yt", [D, C], mybir.dt.float32, isOutput=True)

    HHALF = HC // 2  # w2 streams in half-tiles of 16 h-chunks

    with tile.TileContext(nc) as tc:
        with (
            tc.tile_pool(name="bias", bufs=1) as bp,
            tc.tile_pool(name="x", bufs=2) as xp,
            tc.tile_pool(name="h", bufs=1) as hp,
            tc.tile_pool(name="w1p", bufs=3) as w1p,
            tc.tile_pool(name="w2p", bufs=3) as w2p,
            tc.tile_pool(name="o", bufs=4) as op,
            tc.tile_pool(name="psA", bufs=4, space="PSUM") as pA,
            tc.tile_pool(name="psB", bufs=4, space="PSUM") as pB,
        ):
            b1t = bp.tile([128, HC], mybir.dt.float32)
            nc.scalar.dma_start(b1t[:], b1_d[:])

            w1_tiles = {}

            def w1_prefetch(hj):
                t = w1p.tile([128, DC * 128], DT, tag="w1")
                nc.sync.dma_start(t[:], w1_d[hj].rearrange("d di h -> d (di h)"))
                w1_tiles[hj] = t

            def load_xt(sbase, sw):
                # column-major so the first column block lands fast
                xt = xp.tile([128, DC * SC], DT, tag="x")
                for cbase, cw in _cols(sw):
                    for di in range(DC):
                        nc.sync.dma_start(
                            xt[:, di * sw + cbase : di * sw + cbase + cw],
                            xt_d[
                                di * 128 : (di + 1) * 128,
                                sbase + cbase : sbase + cbase + cw,
                            ],
                        )
                return xt

            scs = _chunks(C, SC) * repeat
            w1_prefetch(0)  # ahead of the token loads in SP queue order
            xt_next = load_xt(*scs[0])

            for si, (sbase, sw) in enumerate(scs):
                    cols = _cols(sw)
                    xt = xt_next

                    # w2 half-tiles to prefetch this super-chunk, in use order
                    w2_queue = [(dj, hh) for dj in range(DC) for hh in range(2)]
                    w2_tiles = {}

                    def w2_prefetch(n=1):
                        for _ in range(n):
                            if not w2_queue:
                                return
                            dj, hh = w2_queue.pop(0)
                            t = w2p.tile([128, HHALF * 128], HDT, tag="w2")
                            nc.sync.dma_start(
                                t[:],
                                w2_d[
                                    dj, :, hh * HHALF : (hh + 1) * HHALF, :
                                ].rearrange("h hj d -> h (hj d)"),
                            )
                            w2_tiles[(dj, hh)] = t

                    ht = hp.tile([128, HC * SC], HDT, tag="h")

                    # ---- phase A: hT = gelu(W1.T @ xT + b1) ----
                    for hj in range(HC):
                        if hj not in w1_tiles:
                            w1_prefetch(hj)
                        if hj + 1 < HC and hj + 1 not in w1_tiles:
                            w1_prefetch(hj + 1)
                        w1t = w1_tiles.pop(hj)
                        if hj in (1, 11, 21):  # warm the w2 pipeline early
                            w2_prefetch()
                        for cbase, cw in cols:
                            ps = pA.tile([128, cw], mybir.dt.float32, tag="psA")
                            for di in range(DC):
                                nc.tensor.matmul(
                                    ps[:],
                                    w1t[:, di * 128 : (di + 1) * 128],
                                    xt[:, di * sw + cbase : di * sw + cbase + cw],
                                    start=(di == 0),
                                    stop=(di == DC - 1),
                                )
                            nc.scalar.activation(
                                ht[:, hj * sw + cbase : hj * sw + cbase + cw],
                                ps[:],
                                mybir.ActivationFunctionType.Gelu,
                                bias=b1t[:, hj : hj + 1],
                            )

                    # ---- phase B: yT = W2.T @ hT (stores straight from PSUM)
                    for dj in range(DC):
                        if dj == 1 and si + 1 < len(scs):
                            # prefetch next super-chunk tokens + first weights
                            # while PE is busy
                            xt_next = load_xt(*scs[si + 1])
                            w1_prefetch(0)
                        for cbase, cw in cols:
                            ps2 = pB.tile([128, cw], mybir.dt.float32, tag="psB")
                            for hj in range(HC):
                                if hj % HHALF == 0 and cbase == 0:
                                    w2_prefetch()
                                w2t = w2_tiles[(dj, hj // HHALF)]
                                nc.tensor.matmul(
                                    ps2[:],
                                    w2t[
                                        :,
                                        (hj % HHALF) * 128 : (hj % HHALF + 1) * 128,
                                    ],
                                    ht[:, hj * sw + cbase : hj * sw + cbase + cw],
                                    start=(hj == 0),
                                    stop=(hj == HC - 1),
                                )
                            ot = op.tile([128, cw], mybir.dt.float32, tag="o")
                            nc.vector.tensor_copy(ot[:], ps2[:])
                            nc.scalar.dma_start(
                                yt_d[
                                    dj * 128 : (dj + 1) * 128,
                                    sbase + cbase : sbase + cbase + cw,
                                ],
                                ot[:],
                            )
    nc.compile()
    _cache[key] = nc
    return nc


def _route(xf, Wg):
    """Host router: softmax top-2, normalized weights, aux loss."""
    logits = xf @ Wg  # [T, E] fp32
    m = logits.max(axis=1, keepdims=True)
    ex = np.exp(logits - m)
    probs = ex / ex.sum(axis=1, keepdims=True)
    idx = np.argsort(-probs, axis=1, kind="stable")[:, :TOP_K]  # [T, K]
    w = np.take_along_axis(probs, idx, axis=1)
    w = w / w.sum(axis=1, keepdims=True)

    T = xf.shape[0]
    counts = np.zeros((T, E), dtype=np.float32)
    np.put_along_axis(counts, idx, 1.0, axis=1)
    f_i = counts.mean(axis=0)
    p_i = probs.mean(axis=0, dtype=np.float32)
    aux_loss = np.float32(E) * np.sum(f_i * p_i, dtype=np.float32)
    return idx, w.astype(np.float32), aux_loss


def kernel(x, Wg, W1, b1, W2, b2):
    x = np.asarray(x, dtype=np.float32)
    Wg = np.asarray(Wg, dtype=np.float32)
    W1 = np.asarray(W1, dtype=np.float32)
    b1 = np.asarray(b1, dtype=np.float32)
    W2 = np.asarray(W2, dtype=np.float32)
    b2 = np.asarray(b2, dtype=np.float32)

    B, S, _ = x.shape
    T = B * S
    xf = x.reshape(T, D)

    idx, w, aux_loss = _route(xf, Wg)

    # token lists per expert
    toks = [np.where((idx == e).any(axis=1))[0] for e in range(E)]
    n_max = max(len(t) for t in toks)
    C = max(256, -(-n_max // 32) * 32)

    nc = _build(C)

    in_maps = []
    for e in range(E):
        te = toks[e]
        xT = np.zeros((D, C), dtype=np.float32)
        xT[:, : len(te)] = xf[te].T
        w1p = np.ascontiguousarray(
            W1[e].reshape(DC, 128, HC, 128).transpose(2, 1, 0, 3)
        )
        w2p = np.ascontiguousarray(
            W2[e].reshape(HC, 128, DC, 128).transpose(2, 1, 0, 3)
        )
        if BF16_H:
            import ml_dtypes

            w2p = w2p.astype(ml_dtypes.bfloat16)
        b1p = np.ascontiguousarray(b1[e].reshape(HC, 128).T)
        in_maps.append({"xt": xT, "w1": w1p, "b1": b1p, "w2": w2p})

    res = run_bass_kernel_spmd(nc, in_maps, core_ids=list(range(E)))

    out = np.zeros((T, D), dtype=np.float32)
    for e in range(E):
        te = toks[e]
        if len(te) == 0:
            continue
        y_e = res.results[e]["yt"][:, : len(te)].T + b2[e][None, :]  # [n_e, D]
        # weight of expert e for each of its tokens
        sel = idx[te] == e  # [n_e, K] one-hot over K
        w_e = (w[te] * sel).sum(axis=1).astype(np.float32)  # [n_e]
        out[te] += w_e[:, None] * y_e

    return out.reshape(B, S, D), aux_loss
